# revision 44
# baseline (speedup 1.0000x reference)
"""Trainium2 Bass kernel for ConvGPTAttention (dense transformer attention block).

Sharding: tensor-parallel by head groups across 8 NeuronCores.
Core j owns q heads {2j, 2j+1} and kv head j (GQA maps q head h -> kv head h//2,
so each core's attention is fully local). Wqkv is column-sharded, Wo is
row-sharded; the 8 partial o_proj outputs are summed on the host (the
"all-reduce" of RowParallelLinear, done at unshard time).

Per-core pipeline:
  Phase A: qkv = X @ Wqkv_shard via fp8 DoubleRow matmuls (2 cols/PE-cycle),
           3-term error compensation: X4=4X, W32=32W split into e4m3 main +
           e5m2 residual; qkv = X8@W8 + Xr@W8 + X8@Wr, PSUM carries 128x the
           true value and the post-copies fold in 1/128. Per-head RMSNorm
           stats via tensor_tensor_reduce (DVE), neox RoPE on fp16 operands
           (2x DVE modes), PE transposes of q/k in fp16, all matmul operands
           in fp16 thereafter.
  Phase B: causal attention per (q head, 512-col t-block) in S^T layout at
           s-block (128-token) granularity; exp on ACT (softmax scale folded),
           av/sum-exp matmuls lag two s-blocks behind S^T; sum-exp ones
           vector holds 1/16 so attn is normalized to 16*attn (keeps the
           on-device e4m3 split of attn out of the subnormal range).
           o_proj in fp8 DoubleRow with 3-term compensation (attn8/attnr
           split on DVE, Wo8/Wor split on host, output copies scale 1/256);
           o_proj partial outputs are batched per 128-token row block into
           single DMAs.
"""

import numpy as np
from contextlib import ExitStack

import concourse.bacc as bacc
import concourse.mybir as mybir
import concourse.tile as tile
from concourse.bass_utils import run_bass_kernel_spmd

P = 128
T = 2048
H = 2048
N_HEADS = 16
N_KV = 8
HD = 128
EPS = 1e-6
THETA = 10000.0
SCALE = HD ** -0.5
NEG = -60000.0  # additive mask (fp16-safe); SCALE*NEG = -5303 -> exp == 0.0

F32 = mybir.dt.float32
F16 = mybir.dt.float16
F8E4 = mybir.dt.float8e4
F8E5 = mybir.dt.float8e5
AF = mybir.ActivationFunctionType
ALU = mybir.AluOpType
DR = mybir.MatmulPerfMode.DoubleRow

N_CORES = 8
N_TT = 16        # t-tiles of 128 tokens
N_TB = 4         # t-blocks of 512 tokens (attention rhs width)


def _build_nc():
    nc = bacc.Bacc("TRN2", target_bir_lowering=False, debug=False)

    x8 = nc.dram_tensor("x8", [H, T], F8E4, kind="ExternalInput")
    xr = nc.dram_tensor("xr", [H, T], F8E5, kind="ExternalInput")
    w8 = nc.dram_tensor("w8", [H, 512], F8E4, kind="ExternalInput")
    wr = nc.dram_tensor("wr", [H, 512], F8E5, kind="ExternalInput")
    wo8 = nc.dram_tensor("wo8", [256, H], F8E4, kind="ExternalInput")
    wor = nc.dram_tensor("wor", [256, H], F8E5, kind="ExternalInput")
    ab = nc.dram_tensor("ab", [T, 4, 3, 64], F16, kind="ExternalInput")
    maskc = nc.dram_tensor("maskc", [P, 4, 512], F16, kind="ExternalInput")
    ident = nc.dram_tensor("ident", [P, P], F16, kind="ExternalInput")
    wo16 = nc.dram_tensor("wo16", [256, H], F16, kind="ExternalInput")
    out = nc.dram_tensor("out", [T, H], F16, kind="ExternalOutput")

    with ExitStack() as top:
        tc = top.enter_context(tile.TileContext(nc))
        pers = top.enter_context(tc.tile_pool(name="pers", bufs=1))

        mask_sb = pers.tile([P, 4, 512], F16, tag="maskc")
        ident_sb = pers.tile([P, P], F16, tag="ident")
        ones_sb = pers.tile([P, 1], F16, tag="ones")
        nc.vector.memset(ones_sb[:], 1.0 / 16.0)  # folds 1/16 into sum-exp
        # DoubleRow matmuls must target PSUM partition 0, so one joint DR
        # sum-exp serves both heads: slot 0 = head0 es, slot 1 = head1 es,
        # and a structured ones matrix routes slot 0 sums to rows [0,32)
        # and slot 1 sums to rows [64,96). Only rows 0 and 64 are read.
        ones8_sb = pers.tile([P, 2, P], F8E4, tag="ones8")
        nc.vector.memset(ones8_sb[:], 0.0)
        nc.vector.memset(ones8_sb[:, 0, 0:32], 1.0 / 16.0)  # 2^-4: exact
        nc.vector.memset(ones8_sb[:, 1, 64:96], 1.0 / 16.0)
        eps_sb = pers.tile([P, 1], F32, tag="eps")
        nc.vector.memset(eps_sb[:], EPS)

        # persistent activations (all fp16 matmul operands)
        qT = pers.tile([P, 3, T], F16, tag="qT")        # [d, (q0,q1,k), t]
        v_tok = pers.tile([P, N_TT, P], F16, tag="v")   # [t_in, tt, d]
        attn8 = pers.tile([P, 2, T], F8E4, tag="attn8")  # 16*attn, e4m3 part
        attnr = pers.tile([P, 2, T], F8E5, tag="attnr")  # residual
        w8_sb = pers.tile([P, 16, 512], F8E4, tag="w8")
        wr_sb = pers.tile([P, 16, 512], F8E5, tag="wr")
        wo8_sb = pers.tile([P, 2, H], F8E4, tag="wo8")
        wor_sb = pers.tile([P, 2, H], F8E5, tag="wor")
        wo16_sb = pers.tile([P, 2, H], F16, tag="wo16")
        attn16 = pers.tile([P, 2, 512], F16, tag="attn16")  # tb=3: 16*attn fp16

        # ---------------- Phase A: QKV + norm + rope + transpose ------------
        with ExitStack() as pa_ctx:
            xtp = pa_ctx.enter_context(tc.tile_pool(name="xtp", bufs=1))
            pa = pa_ctx.enter_context(tc.tile_pool(name="pa", bufs=3))
            psa = pa_ctx.enter_context(tc.tile_pool(name="psa", bufs=2, space="PSUM"))
            pst = pa_ctx.enter_context(tc.tile_pool(name="pst", bufs=2, space="PSUM"))

            scr = xtp.tile([P, 2, 512], F8E4, tag="scr")
            nc.vector.memset(scr[:], 0.0)
            ps_w = psa.tile([P, 2, 512], F32, tag="psa", name="ps_warm")
            for i in range(12):
                nc.tensor.matmul(
                    ps_w[:, i % 2, :],
                    scr[:, :, 0:128],
                    scr[:],
                    start=True,
                    stop=True,
                    perf_mode=DR,
                    skip_group_check=True,
                )

            # X panels live in SBUF for the whole phase (64 KiB/partition):
            # every DMA is issued once, up front, across both HWDGE queues
            x8_sb = xtp.tile([P, 16, T], F8E4, tag="x8")
            xr_sb = xtp.tile([P, 16, T], F8E5, tag="xr")
            ab_sb = xtp.tile([P, 16, 4, 3, 64], F16, tag="ab")

            def dma_w(dst, src, chunks):
                step = 16 // chunks
                for c in range(chunks):
                    nc.sync.dma_start(
                        dst[:, c * step:(c + 1) * step, :],
                        src.rearrange("(hi p) n -> p hi n", p=P)[
                            :, c * step:(c + 1) * step, :
                        ],
                    )

            def dma_x(dst, src, tsb, chunks):
                step = 16 // chunks
                srcv = src.rearrange("(hi p) n -> p hi n", p=P)
                cols = slice(tsb * 512, (tsb + 1) * 512)
                for c in range(chunks):
                    nc.sync.dma_start(
                        dst[:, c * step:(c + 1) * step, cols],
                        srcv[:, c * step:(c + 1) * step, cols],
                    )

            def post_rope(tg, tgl, ps_a, ab_sb):
                # views on the psum group: [P, 2(ttl), 4(head), 128(d)]
                # psum holds 128*qkv (X scaled 4x, W scaled 32x)
                ps_r = ps_a.rearrange("p g (h d) -> p g h d", h=4)

                # raw q/k in fp16 (scale back), v straight to persistent
                qkn_raw = pa.tile([P, 2, 3, 128], F16, tag="qkraw")
                nc.scalar.activation(
                    qkn_raw[:], ps_r[:, :, 0:3, :], AF.Copy, scale=1.0 / 128.0
                )
                nc.scalar.activation(
                    v_tok[:, 2 * tg:2 * tg + 2, :], ps_r[:, :, 3, :], AF.Copy,
                    scale=1.0 / 128.0,
                )

                # RMS stats: square (DVE, 2x fp16) -> free-axis reduce
                sq = pa.tile([P, 2, 3, 128], F16, tag="sq")
                nc.vector.tensor_mul(sq[:], qkn_raw[:], qkn_raw[:])
                ss = pa.tile([P, 2, 3], F32, tag="ss")
                nc.vector.tensor_reduce(
                    ss[:], sq[:], axis=mybir.AxisListType.X, op=ALU.add
                )
                sr = pa.tile([P, 2, 3], F32, tag="sr")
                nc.scalar.activation(
                    sr[:], ss[:], AF.Sqrt, scale=1.0 / HD, bias=eps_sb[:]
                )
                s_inv = pa.tile([P, 2, 3], F32, tag="si")
                nc.vector.reciprocal(s_inv[:], sr[:])

                # rope (tables have norm weight folded in, heads packed
                # (q0, q1, k) along the table's head dim):
                # out1 = x1*a1 - x2*b1 ; out2 = x2*a2 + x1*b2
                qkn = pa.tile([P, 2, 3, 128], F16, tag="qkn")
                x1 = qkn_raw[:, :, :, 0:64]
                x2 = qkn_raw[:, :, :, 64:128]
                abg = ab_sb[:, 2 * tg:2 * tg + 2]    # [P, 2, 4, 3, 64]
                m1 = pa.tile([P, 2, 3, 64], F16, tag="m1")
                m2 = pa.tile([P, 2, 3, 64], F16, tag="m2")
                nc.vector.tensor_mul(m1[:], x1, abg[:, :, 0])
                nc.vector.tensor_mul(m2[:], x2, abg[:, :, 1])
                nc.vector.tensor_sub(qkn[:, :, :, 0:64], m1[:], m2[:])
                nc.vector.tensor_mul(m1[:], x2, abg[:, :, 2])
                nc.vector.tensor_mul(m2[:], x1, abg[:, :, 3])
                nc.vector.tensor_add(qkn[:, :, :, 64:128], m1[:], m2[:])

                # apply 1/rms (per token+head scalar, 2x DVE mode kept)
                for k in range(6):
                    ttl, h = divmod(k, 3)
                    nc.vector.tensor_scalar_mul(
                        qkn[:, ttl, h, :], qkn[:, ttl, h, :],
                        s_inv[:, ttl, h:h + 1],
                    )
                return qkn

            def post_transpose(tg, qkn):
                # transpose q0/q1/k to [d, t] (fp16 transposes, 1 cyc/row)
                for ttl in range(2):
                    tt = 2 * tg + ttl
                    ps_t = pst.tile([P, 3, P], F16, tag="pst")
                    for h in range(3):
                        nc.tensor.transpose(
                            ps_t[:, h, :], qkn[:, ttl, h, :], ident_sb[:]
                        )
                    nc.vector.tensor_copy(
                        qT[:, :, tt * 128:(tt + 1) * 128], ps_t[:]
                    )

            def emit_chain(ps_a, ttl, x8_sb, xr_sb, tsb, tgl):
                c0 = tsb * 512 + tgl * 256 + ttl * 128
                col = slice(c0, c0 + 128)
                idx = 0
                for xs, ws in ((x8_sb, w8_sb), (x8_sb, wr_sb), (xr_sb, w8_sb)):
                    for j in range(8):
                        nc.tensor.matmul(
                            ps_a[:, ttl, :],
                            xs[:, 2 * j:2 * j + 2, col],
                            ws[:, 2 * j:2 * j + 2, :],
                            start=(idx == 0),
                            stop=(idx == 23),
                            perf_mode=DR,
                        )
                        idx += 1

            # --- all input DMAs up front, on the SP queue, ordered by
            # first-consumption time (issue cadence is ~0.9us/DMA, so the
            # order IS the arrival schedule)
            w8v = w8.rearrange("(hi p) n -> p hi n", p=P)
            x8v = x8.rearrange("(hi p) n -> p hi n", p=P)
            abv = ab.rearrange("(tt p) f h d -> p tt f h d", p=P)
            for c in range(4):
                hs = slice(c * 4, (c + 1) * 4)
                nc.sync.dma_start(w8_sb[:, hs, :], w8v[:, hs, :])
                nc.sync.dma_start(x8_sb[:, hs, 0:512], x8v[:, hs, 0:512])
            dma_w(wr_sb, wr, 2)
            dma_x(xr_sb, xr, 0, 2)
            nc.sync.dma_start(ab_sb[:, 0:8], abv[:, 0:8])
            dma_x(x8_sb, x8, 1, 1)
            dma_x(xr_sb, xr, 1, 1)
            nc.sync.dma_start(ab_sb[:, 8:16], abv[:, 8:16])
            dma_x(x8_sb, x8, 2, 1)
            dma_x(xr_sb, xr, 2, 1)
            nc.sync.dma_start(ident_sb[:], ident[:])
            dma_x(x8_sb, x8, 3, 1)
            dma_x(xr_sb, xr, 3, 1)
            nc.sync.dma_start(
                wo8_sb[:], wo8.rearrange("(do p) h -> p do h", p=P)
            )
            nc.sync.dma_start(
                wor_sb[:], wor.rearrange("(do p) h -> p do h", p=P)
            )
            nc.sync.dma_start(
                wo16_sb[:], wo16.rearrange("(do p) h -> p do h", p=P)
            )
            nc.sync.dma_start(mask_sb[:], maskc[:])

            pending = []  # (tg, qkn) awaiting transposes (lag 2 chains)
            for tsb in range(4):          # 512-token superblocks (X^T panels)
                for tgl in range(2):  # groups of 2 t-tiles (256 tokens)
                    tg = tsb * 2 + tgl
                    ps_a = psa.tile([P, 2, 512], F32, tag="psa")
                    for ttl in range(2):
                        emit_chain(ps_a, ttl, x8_sb, xr_sb, tsb, tgl)
                    if len(pending) >= 2:
                        post_transpose(*pending.pop(0))
                    qkn = post_rope(tg, tgl, ps_a, ab_sb)
                    pending.append((tg, qkn))
            for item in pending:
                post_transpose(*item)

        # ---------------- Phase B: attention + o_proj -----------------------
        with ExitStack() as pb_ctx:
            expp = pb_ctx.enter_context(tc.tile_pool(name="expp", bufs=8))
            nrm = pb_ctx.enter_context(tc.tile_pool(name="nrm", bufs=2))
            esq = pb_ctx.enter_context(tc.tile_pool(name="esq", bufs=4))
            outp = pb_ctx.enter_context(tc.tile_pool(name="outp", bufs=4))
            pss = pb_ctx.enter_context(tc.tile_pool(name="pss", bufs=3, space="PSUM"))
            psat = pb_ctx.enter_context(tc.tile_pool(name="psat", bufs=2, space="PSUM"))
            psse = pb_ctx.enter_context(tc.tile_pool(name="psse", bufs=1, space="PSUM"))
            pso = pb_ctx.enter_context(tc.tile_pool(name="pso", bufs=2, space="PSUM"))

            out_tiles = {}
            flush_ctr = [0]

            def emit_oproj_unit(tt, hb, flush=False, eng="mix"):
                if hb == 0:
                    out_tiles[tt] = outp.tile(
                        [P, H], F16, tag="osb", name=f"osb_{tt}"
                    )
                o_sb = out_tiles[tt]
                if flush:
                    # during a flush the attention PSUM pools are idle:
                    # rotate across them so the PE never waits on a copy.
                    # mid-run flushes avoid psat (normalize still reads it).
                    pools = ((pso, "o"), (pss, "st"), (psat, "at"))[:flush]
                    pool, tg = pools[flush_ctr[0] % len(pools)]
                    flush_ctr[0] += 1
                    ps_o = pool.tile(
                        [P, 512], F32, tag=tg, name=f"of_{tt}_{hb}"
                    )
                else:
                    ps_o = pso.tile([P, 512], F32, tag="o")
                tcol = slice(tt * 128, (tt + 1) * 128)
                hcol = slice(hb * 512, (hb + 1) * 512)
                if tt >= 12:
                    # last t-block: plain fp16 o_proj (attn16 is ready right
                    # after the normalize mul -- no fp8 split on the tail)
                    lc = slice((tt - 12) * 128, (tt - 11) * 128)
                    for hh in range(2):
                        nc.tensor.matmul(
                            ps_o[:],
                            attn16[:, hh, lc],
                            wo16_sb[:, hh, hcol],
                            start=(hh == 0),
                            stop=(hh == 1),
                        )
                    oscale = 1.0 / 16.0  # attn carried 16x, wo16 exact
                else:
                    for i, (a, w) in enumerate(
                        ((attn8, wo8_sb), (attnr, wo8_sb), (attn8, wor_sb))
                    ):
                        nc.tensor.matmul(
                            ps_o[:],
                            a[:, :, tcol],
                            w[:, :, hcol],
                            start=(i == 0),
                            stop=(i == 2),
                            perf_mode=DR,
                        )
                    oscale = 1.0 / 256.0  # attn carried 16x, Wo carried 16x
                if (flush and hb % 2 == 0) or (not flush and eng == "mix" and hb % 2 == 0):
                    nc.scalar.activation(
                        o_sb[:, hcol], ps_o[:], AF.Copy, scale=oscale
                    )
                else:
                    nc.vector.tensor_scalar_mul(o_sb[:, hcol], ps_o[:], oscale)
                if hb == 3:
                    nc.sync.dma_start(
                        out[tt * 128:(tt + 1) * 128, :], o_sb[:]
                    )

            # o_proj units of t-block tbo, woven into the next attention
            # t-block's PE stream
            oproj_queue = []

            def queue_oproj(tbo):
                for ttl in range(4):
                    for hb in range(4):
                        oproj_queue.append((4 * tbo + ttl, hb))

            def normalize(qh, c0, width, ps_at, bc, fast=False):
                # attn_full = 16*attn (ones=1/16 in sum-exp); split into
                # e4m3 main + e5m2 residual for the fp8 o_proj
                s_ps = slice(c0 % 512, c0 % 512 + width)
                s_at = slice(c0, c0 + width)
                if fast:
                    # last t-block: straight to the fp16 o_proj operand
                    nc.vector.tensor_mul(
                        attn16[:, qh, s_ps], ps_at[:, s_ps], bc[:, s_ps]
                    )
                    return
                afull = nrm.tile([P, 512], F16, tag="afull")
                nc.vector.tensor_mul(
                    afull[:, 0:width], ps_at[:, s_ps], bc[:, s_ps]
                )
                nc.vector.tensor_copy(attn8[:, qh, s_at], afull[:, 0:width])
                nc.vector.tensor_sub(
                    attnr[:, qh, s_at], afull[:, 0:width], attn8[:, qh, s_at]
                )

            for tb in range(N_TB):
                qrhs0 = tb * 512
                nsb = 4 * (tb + 1)    # s-blocks of 128 tokens
                # diagonal (masked) s-blocks first (their longer
                # S^T -> mask -> exp chain pipelines over later blocks);
                # the two q heads interleave so av/se for head A fill the
                # PE while head B's exp chain cooks (and vice versa).
                diag = list(range(4 * tb, 4 * tb + 4))
                rest = list(range(4 * tb))
                sb_order = []
                for i in range(max(len(diag), len(rest))):
                    if i < len(diag):
                        sb_order.append(diag[i])
                    if i < len(rest):
                        sb_order.append(rest[i])

                ps_at = [psat.tile([P, 512], F32, tag="at", name=f"at{q}")
                         for q in range(2)]
                # one shared [P,512] bank: head 0 sums on partition 0,
                # head 1 on partition 32 (matmul outputs must start at a
                # multiple of 32)
                ps_se = psse.tile([P, 512], F32, tag="se")
                se_row = {0: 0, 1: 64}
                first_se = {0: True, 1: True}
                use_dr_se = tb in (1, 2)  # tb=3 saturates the Pool engine
                # countdown of SE matmuls: 4 diag per head + 1 joint DR (or
                # 2 fp16) per off-diag block
                se_left = [8 + (4 * tb if use_dr_se else 8 * tb)]
                es8_map = {}

                def emit_av(qh, sb, si, o):
                    es = es_tiles[(qh, sb)]
                    r = se_row[qh]
                    nc.tensor.matmul(
                        ps_at[qh][:, o:512],
                        v_tok[:, sb, :],
                        es[:, o:512],
                        start=(si == 0),
                        stop=(si == nsb - 1),
                        skip_group_check=True,
                    )
                    if sb >= 4 * tb or not use_dr_se:
                        # diagonal: exact fp16 sum-exp (keeps early rows'
                        # denominators quantization-free)
                        se_left[0] -= 1
                        nc.tensor.matmul(
                            ps_se[r:r + 1, o:512],
                            ones_sb[:],
                            es[:, o:512],
                            start=first_se[qh],
                            stop=(se_left[0] == 0),
                            skip_group_check=True,
                        )
                        first_se[qh] = False
                    elif qh == 1:
                        # off-diagonal: one joint DR matmul sums both heads'
                        # 128 s-rows (rows 1-31/65-95 accumulate garbage on
                        # never-started psum; only rows 0 and 64 are read)
                        pair = es8_map[sb]
                        se_left[0] -= 1
                        nc.tensor.matmul(
                            ps_se[:, :],
                            ones8_sb[:],
                            pair[:],
                            start=False,
                            stop=(se_left[0] == 0),
                            perf_mode=DR,
                            skip_group_check=True,
                        )

                es_tiles = {}
                pending = []
                for si, sb in enumerate(sb_order):
                    off = sb - 4 * tb
                    # diagonal blocks: columns left of the triangle are
                    # fully causally masked -- skip them outright
                    o = 128 * off if 0 <= off < 4 else 0
                    for qh in range(2):
                        ps_s = pss.tile([P, 512], F32, tag="st")
                        nc.tensor.matmul(
                            ps_s[:, o:512],
                            qT[:, 2, sb * 128:(sb + 1) * 128],
                            qT[:, qh, qrhs0 + o:qrhs0 + 512],
                            start=True,
                            stop=True,
                        )
                        if 0 <= off < 4:
                            nc.vector.tensor_add(
                                ps_s[:, o:o + 128], ps_s[:, o:o + 128],
                                mask_sb[:, off, o:o + 128],
                            )
                        es = expp.tile([P, 512], F16, tag="es")
                        es_tiles[(qh, sb)] = es
                        nc.scalar.activation(
                            es[:, o:512], ps_s[:, o:512], AF.Exp, scale=SCALE
                        )
                        if sb < 4 * tb and use_dr_se:
                            if qh == 0:
                                es8_map[sb] = esq.tile(
                                    [P, 2, 512], F8E4, tag="es8",
                                    name=f"es8_{tb}_{si}",
                                )
                            nc.gpsimd.tensor_copy(
                                es8_map[sb][:, qh, :], es[:]
                            )
                        pending.append((qh, sb, si, o))
                        if len(pending) > 4:
                            emit_av(*pending.pop(0))
                    if oproj_queue:
                        weng = "mix" if tb == 1 else "dve"
                        emit_oproj_unit(*oproj_queue.pop(0), eng=weng)
                        if len(oproj_queue) > 10:
                            emit_oproj_unit(*oproj_queue.pop(0), eng=weng)
                def finish_head(qh):
                    r = se_row[qh]
                    inv_sb = nrm.tile([1, 512], F32, tag="inv")
                    nc.vector.reciprocal(inv_sb[:], ps_se[r:r + 1, :])
                    bc = nrm.tile([P, 512], F32, tag="bc")
                    nc.gpsimd.partition_broadcast(bc[:], inv_sb[0:1, :])
                    if tb == N_TB - 1:
                        for ttn in range(4):
                            normalize(qh, qrhs0 + ttn * 128, 128, ps_at[qh],
                                      bc, fast=True)
                    else:
                        normalize(qh, qrhs0, 512, ps_at[qh], bc)

                done_heads = set()
                for item in pending:
                    emit_av(*item)
                    qh_i = item[0]
                    if all(p[0] != qh_i for p in pending[pending.index(item) + 1:]):
                        if qh_i not in done_heads:
                            done_heads.add(qh_i)
                            finish_head(qh_i)
                for qh_i in range(2):
                    if qh_i not in done_heads:
                        finish_head(qh_i)
                while oproj_queue:
                    emit_oproj_unit(*oproj_queue.pop(0), flush=1)
                queue_oproj(tb)
            while oproj_queue:
                emit_oproj_unit(*oproj_queue.pop(0), flush=3)

    nc.compile()
    return nc


_NC_CACHE = {}


def _get_nc():
    if "nc" not in _NC_CACHE:
        _NC_CACHE["nc"] = _build_nc()
    return _NC_CACHE["nc"]


def _fp8_split(x, scale):
    """Split scale*x into an e4m3 main part (subnormals flushed so host and
    PE agree) and an e5m2 residual."""
    import ml_dtypes

    xs = np.asarray(x, dtype=np.float32) * scale
    hi = xs.astype(ml_dtypes.float8_e4m3).astype(np.float32)
    hi[np.abs(hi) < 2.0 ** -6] = 0.0
    lo = (xs - hi).astype(ml_dtypes.float8_e5m2)
    return hi.astype(ml_dtypes.float8_e4m3), lo


def kernel(positions, hidden_states, Wqkv, Wo, q_norm_w, k_norm_w):
    positions = np.asarray(positions)
    out_dtype = np.asarray(hidden_states).dtype
    hs = np.asarray(hidden_states, dtype=np.float32)
    Wqkv = np.asarray(Wqkv, dtype=np.float32)
    Wo = np.asarray(Wo, dtype=np.float32)
    qw = np.asarray(q_norm_w, dtype=np.float32)
    kw = np.asarray(k_norm_w, dtype=np.float32)

    # ----- host-side input prep -----
    xt = np.ascontiguousarray(hs.T)
    x8_h, xr_h = _fp8_split(xt, 4.0)

    inv_freq = (1.0 / (THETA ** (np.arange(0, HD, 2, dtype=np.float32) / HD))).astype(
        np.float32
    )
    freqs = positions.astype(np.float32)[:, None] * inv_freq[None, :]  # [T, 64]
    cos = np.cos(freqs).astype(np.float32)
    sin = np.sin(freqs).astype(np.float32)

    def ab_tables(wvec):
        a1 = cos * wvec[None, :64]
        b1 = sin * wvec[None, 64:]
        a2 = cos * wvec[None, 64:]
        b2 = sin * wvec[None, :64]
        return np.stack([a1, b1, a2, b2], axis=1)  # [T, 4, 64]

    abq = ab_tables(qw)
    abk = ab_tables(kw)
    # combined per-head tables, head order (q0, q1, k)
    ab = np.ascontiguousarray(
        np.stack([abq, abq, abk], axis=2), dtype=np.float16
    )  # [T, 4, 3, 64]

    # causal masks for the four diagonal 128-row groups of each 512-col t-block
    t_in = np.arange(512)
    s_in = np.arange(128)
    maskc = np.empty((P, 4, 512), dtype=np.float16)
    for j in range(4):
        maskc[:, j, :] = np.where(
            (j * 128 + s_in)[:, None] <= t_in[None, :], 0.0, NEG
        )
    ident = np.eye(P, dtype=np.float16)

    q_size = N_HEADS * HD
    kv_size = N_KV * HD
    in_maps = []
    for j in range(N_CORES):
        qs = slice(2 * j * HD, (2 * j + 2) * HD)
        ks = slice(q_size + j * HD, q_size + (j + 1) * HD)
        vs = slice(q_size + kv_size + j * HD, q_size + kv_size + (j + 1) * HD)
        wj = np.ascontiguousarray(
            np.concatenate([Wqkv[:, qs], Wqkv[:, ks], Wqkv[:, vs]], axis=1)
        )
        w8_h, wr_h = _fp8_split(wj, 32.0)
        wo8_h, wor_h = _fp8_split(Wo[qs, :], 16.0)
        in_maps.append(
            {
                "x8": x8_h,
                "xr": xr_h,
                "w8": w8_h,
                "wr": wr_h,
                "wo8": wo8_h,
                "wor": wor_h,
                "wo16": np.ascontiguousarray(Wo[qs, :]).astype(np.float16),
                "ab": ab,
                "maskc": maskc,
                "ident": ident,
            }
        )

    nc = _get_nc()
    res = run_bass_kernel_spmd(nc, in_maps, core_ids=list(range(N_CORES)))

    acc = res.results[0]["out"].astype(np.float32)
    for j in range(1, N_CORES):
        acc += res.results[j]["out"].astype(np.float32)
    return acc.astype(out_dtype, copy=False)


# revision 45
# speedup vs baseline: 1.0070x; 1.0070x over previous
"""Trainium2 Bass kernel for ConvGPTAttention (dense transformer attention block).

Sharding: tensor-parallel by head groups across 8 NeuronCores.
Core j owns q heads {2j, 2j+1} and kv head j (GQA maps q head h -> kv head h//2,
so each core's attention is fully local). Wqkv is column-sharded, Wo is
row-sharded; the 8 partial o_proj outputs are summed on the host (the
"all-reduce" of RowParallelLinear, done at unshard time).

Per-core pipeline:
  Phase A: qkv = X @ Wqkv_shard via fp8 DoubleRow matmuls (2 cols/PE-cycle),
           3-term error compensation: X4=4X, W32=32W split into e4m3 main +
           e5m2 residual; qkv = X8@W8 + Xr@W8 + X8@Wr, PSUM carries 128x the
           true value and the post-copies fold in 1/128. Per-head RMSNorm
           stats via tensor_tensor_reduce (DVE), neox RoPE on fp16 operands
           (2x DVE modes), PE transposes of q/k in fp16, all matmul operands
           in fp16 thereafter.
  Phase B: causal attention per (q head, 512-col t-block) in S^T layout at
           s-block (128-token) granularity; exp on ACT (softmax scale folded),
           av/sum-exp matmuls lag two s-blocks behind S^T; sum-exp ones
           vector holds 1/16 so attn is normalized to 16*attn (keeps the
           on-device e4m3 split of attn out of the subnormal range).
           o_proj in fp8 DoubleRow with 3-term compensation (attn8/attnr
           split on DVE, Wo8/Wor split on host, output copies scale 1/256);
           o_proj partial outputs are batched per 128-token row block into
           single DMAs.
"""

import numpy as np
from contextlib import ExitStack

import concourse.bacc as bacc
import concourse.mybir as mybir
import concourse.tile as tile
from concourse.bass_utils import run_bass_kernel_spmd

P = 128
T = 2048
H = 2048
N_HEADS = 16
N_KV = 8
HD = 128
EPS = 1e-6
THETA = 10000.0
SCALE = HD ** -0.5
NEG = -60000.0  # additive mask (fp16-safe); SCALE*NEG = -5303 -> exp == 0.0

F32 = mybir.dt.float32
F16 = mybir.dt.float16
F8E4 = mybir.dt.float8e4
F8E5 = mybir.dt.float8e5
AF = mybir.ActivationFunctionType
ALU = mybir.AluOpType
DR = mybir.MatmulPerfMode.DoubleRow

N_CORES = 8
N_TT = 16        # t-tiles of 128 tokens
N_TB = 4         # t-blocks of 512 tokens (attention rhs width)


def _build_nc():
    nc = bacc.Bacc("TRN2", target_bir_lowering=False, debug=False)

    x8 = nc.dram_tensor("x8", [H, T], F8E4, kind="ExternalInput")
    xr = nc.dram_tensor("xr", [H, T], F8E5, kind="ExternalInput")
    w8 = nc.dram_tensor("w8", [H, 512], F8E4, kind="ExternalInput")
    wr = nc.dram_tensor("wr", [H, 512], F8E5, kind="ExternalInput")
    wo8 = nc.dram_tensor("wo8", [256, H], F8E4, kind="ExternalInput")
    wor = nc.dram_tensor("wor", [256, H], F8E5, kind="ExternalInput")
    ab = nc.dram_tensor("ab", [T, 4, 3, 64], F16, kind="ExternalInput")
    maskc = nc.dram_tensor("maskc", [P, 4, 512], F16, kind="ExternalInput")
    ident = nc.dram_tensor("ident", [P, P], F16, kind="ExternalInput")
    wo16 = nc.dram_tensor("wo16", [256, H], F16, kind="ExternalInput")
    out = nc.dram_tensor("out", [T, H], F16, kind="ExternalOutput")

    with ExitStack() as top:
        tc = top.enter_context(tile.TileContext(nc))
        pers = top.enter_context(tc.tile_pool(name="pers", bufs=1))

        mask_sb = pers.tile([P, 4, 512], F16, tag="maskc")
        ident_sb = pers.tile([P, P], F16, tag="ident")
        ones_sb = pers.tile([P, 1], F16, tag="ones")
        nc.vector.memset(ones_sb[:], 1.0 / 16.0)  # folds 1/16 into sum-exp
        # DoubleRow matmuls must target PSUM partition 0, so one joint DR
        # sum-exp serves both heads: slot 0 = head0 es, slot 1 = head1 es,
        # and a structured ones matrix routes slot 0 sums to rows [0,32)
        # and slot 1 sums to rows [64,96). Only rows 0 and 64 are read.
        ones8_sb = pers.tile([P, 2, P], F8E4, tag="ones8")
        nc.vector.memset(ones8_sb[:], 0.0)
        nc.vector.memset(ones8_sb[:, 0, 0:32], 1.0 / 16.0)  # 2^-4: exact
        nc.vector.memset(ones8_sb[:, 1, 64:96], 1.0 / 16.0)
        eps_sb = pers.tile([P, 1], F32, tag="eps")
        nc.vector.memset(eps_sb[:], EPS)

        # persistent activations (all fp16 matmul operands)
        qT = pers.tile([P, 3, T], F16, tag="qT")        # [d, (q0,q1,k), t]
        v_tok = pers.tile([P, N_TT, P], F16, tag="v")   # [t_in, tt, d]
        attn8 = pers.tile([P, 2, T], F8E4, tag="attn8")  # 16*attn, e4m3 part
        attnr = pers.tile([P, 2, T], F8E5, tag="attnr")  # residual
        w8_sb = pers.tile([P, 16, 512], F8E4, tag="w8")
        wr_sb = pers.tile([P, 16, 512], F8E5, tag="wr")
        wo8_sb = pers.tile([P, 2, H], F8E4, tag="wo8")
        wor_sb = pers.tile([P, 2, H], F8E5, tag="wor")
        wo16_sb = pers.tile([P, 2, H], F16, tag="wo16")
        attn16 = pers.tile([P, 2, 512], F16, tag="attn16")  # tb=3: 16*attn fp16

        # ---------------- Phase A: QKV + norm + rope + transpose ------------
        with ExitStack() as pa_ctx:
            xtp = pa_ctx.enter_context(tc.tile_pool(name="xtp", bufs=1))
            pa = pa_ctx.enter_context(tc.tile_pool(name="pa", bufs=3))
            psa = pa_ctx.enter_context(tc.tile_pool(name="psa", bufs=2, space="PSUM"))
            pst = pa_ctx.enter_context(tc.tile_pool(name="pst", bufs=2, space="PSUM"))

            scr = xtp.tile([P, 2, 512], F8E4, tag="scr")
            nc.vector.memset(scr[:], 0.0)
            ps_w = psa.tile([P, 2, 512], F32, tag="psa", name="ps_warm")
            for i in range(12):
                nc.tensor.matmul(
                    ps_w[:, i % 2, :],
                    scr[:, :, 0:128],
                    scr[:],
                    start=True,
                    stop=True,
                    perf_mode=DR,
                    skip_group_check=True,
                )

            # X panels live in SBUF for the whole phase (64 KiB/partition):
            # every DMA is issued once, up front, across both HWDGE queues
            x8_sb = xtp.tile([P, 16, T], F8E4, tag="x8")
            xr_sb = xtp.tile([P, 16, T], F8E5, tag="xr")
            ab_sb = xtp.tile([P, 16, 4, 3, 64], F16, tag="ab")

            def dma_w(dst, src, chunks):
                step = 16 // chunks
                for c in range(chunks):
                    nc.sync.dma_start(
                        dst[:, c * step:(c + 1) * step, :],
                        src.rearrange("(hi p) n -> p hi n", p=P)[
                            :, c * step:(c + 1) * step, :
                        ],
                    )

            def dma_x(dst, src, tsb, chunks):
                step = 16 // chunks
                srcv = src.rearrange("(hi p) n -> p hi n", p=P)
                cols = slice(tsb * 512, (tsb + 1) * 512)
                for c in range(chunks):
                    nc.sync.dma_start(
                        dst[:, c * step:(c + 1) * step, cols],
                        srcv[:, c * step:(c + 1) * step, cols],
                    )

            def post_rope(tg, tgl, ps_a, ab_sb):
                # views on the psum group: [P, 2(ttl), 4(head), 128(d)]
                # psum holds 128*qkv (X scaled 4x, W scaled 32x)
                ps_r = ps_a.rearrange("p g (h d) -> p g h d", h=4)

                # raw q/k in fp16 (scale back), v straight to persistent
                qkn_raw = pa.tile([P, 2, 3, 128], F16, tag="qkraw")
                nc.scalar.activation(
                    qkn_raw[:], ps_r[:, :, 0:3, :], AF.Copy, scale=1.0 / 128.0
                )
                nc.scalar.activation(
                    v_tok[:, 2 * tg:2 * tg + 2, :], ps_r[:, :, 3, :], AF.Copy,
                    scale=1.0 / 128.0,
                )

                # RMS stats: square (DVE, 2x fp16) -> free-axis reduce
                sq = pa.tile([P, 2, 3, 128], F16, tag="sq")
                nc.vector.tensor_mul(sq[:], qkn_raw[:], qkn_raw[:])
                ss = pa.tile([P, 2, 3], F32, tag="ss")
                nc.vector.tensor_reduce(
                    ss[:], sq[:], axis=mybir.AxisListType.X, op=ALU.add
                )
                sr = pa.tile([P, 2, 3], F32, tag="sr")
                nc.scalar.activation(
                    sr[:], ss[:], AF.Sqrt, scale=1.0 / HD, bias=eps_sb[:]
                )
                s_inv = pa.tile([P, 2, 3], F32, tag="si")
                nc.vector.reciprocal(s_inv[:], sr[:])

                # rope (tables have norm weight folded in, heads packed
                # (q0, q1, k) along the table's head dim):
                # out1 = x1*a1 - x2*b1 ; out2 = x2*a2 + x1*b2
                qkn = pa.tile([P, 2, 3, 128], F16, tag="qkn")
                x1 = qkn_raw[:, :, :, 0:64]
                x2 = qkn_raw[:, :, :, 64:128]
                abg = ab_sb[:, 2 * tg:2 * tg + 2]    # [P, 2, 4, 3, 64]
                m1 = pa.tile([P, 2, 3, 64], F16, tag="m1")
                m2 = pa.tile([P, 2, 3, 64], F16, tag="m2")
                nc.vector.tensor_mul(m1[:], x1, abg[:, :, 0])
                nc.vector.tensor_mul(m2[:], x2, abg[:, :, 1])
                nc.vector.tensor_sub(qkn[:, :, :, 0:64], m1[:], m2[:])
                nc.vector.tensor_mul(m1[:], x2, abg[:, :, 2])
                nc.vector.tensor_mul(m2[:], x1, abg[:, :, 3])
                nc.vector.tensor_add(qkn[:, :, :, 64:128], m1[:], m2[:])

                # apply 1/rms (per token+head scalar, 2x DVE mode kept)
                for k in range(6):
                    ttl, h = divmod(k, 3)
                    nc.vector.tensor_scalar_mul(
                        qkn[:, ttl, h, :], qkn[:, ttl, h, :],
                        s_inv[:, ttl, h:h + 1],
                    )
                return qkn

            def post_transpose(tg, qkn):
                # transpose q0/q1/k to [d, t] (fp16 transposes, 1 cyc/row)
                for ttl in range(2):
                    tt = 2 * tg + ttl
                    ps_t = pst.tile([P, 3, P], F16, tag="pst")
                    for h in range(3):
                        nc.tensor.transpose(
                            ps_t[:, h, :], qkn[:, ttl, h, :], ident_sb[:]
                        )
                    nc.vector.tensor_copy(
                        qT[:, :, tt * 128:(tt + 1) * 128], ps_t[:]
                    )

            def emit_chain(ps_a, ttl, x8_sb, xr_sb, tsb, tgl):
                c0 = tsb * 512 + tgl * 256 + ttl * 128
                col = slice(c0, c0 + 128)
                idx = 0
                for xs, ws in ((x8_sb, w8_sb), (x8_sb, wr_sb), (xr_sb, w8_sb)):
                    for j in range(8):
                        nc.tensor.matmul(
                            ps_a[:, ttl, :],
                            xs[:, 2 * j:2 * j + 2, col],
                            ws[:, 2 * j:2 * j + 2, :],
                            start=(idx == 0),
                            stop=(idx == 23),
                            perf_mode=DR,
                        )
                        idx += 1

            # --- all input DMAs up front, on the SP queue, ordered by
            # first-consumption time (issue cadence is ~0.9us/DMA, so the
            # order IS the arrival schedule)
            w8v = w8.rearrange("(hi p) n -> p hi n", p=P)
            x8v = x8.rearrange("(hi p) n -> p hi n", p=P)
            abv = ab.rearrange("(tt p) f h d -> p tt f h d", p=P)
            for c in range(4):
                hs = slice(c * 4, (c + 1) * 4)
                nc.sync.dma_start(w8_sb[:, hs, :], w8v[:, hs, :])
                nc.sync.dma_start(x8_sb[:, hs, 0:512], x8v[:, hs, 0:512])
            dma_w(wr_sb, wr, 2)
            dma_x(xr_sb, xr, 0, 2)
            nc.sync.dma_start(ab_sb[:, 0:8], abv[:, 0:8])
            dma_x(x8_sb, x8, 1, 1)
            dma_x(xr_sb, xr, 1, 1)
            nc.sync.dma_start(ab_sb[:, 8:16], abv[:, 8:16])
            dma_x(x8_sb, x8, 2, 1)
            dma_x(xr_sb, xr, 2, 1)
            nc.sync.dma_start(ident_sb[:], ident[:])
            dma_x(x8_sb, x8, 3, 1)
            dma_x(xr_sb, xr, 3, 1)
            nc.sync.dma_start(
                wo8_sb[:], wo8.rearrange("(do p) h -> p do h", p=P)
            )
            nc.sync.dma_start(
                wor_sb[:], wor.rearrange("(do p) h -> p do h", p=P)
            )
            nc.sync.dma_start(
                wo16_sb[:], wo16.rearrange("(do p) h -> p do h", p=P)
            )
            nc.sync.dma_start(mask_sb[:], maskc[:])

            pending = []  # (tg, qkn) awaiting transposes (lag 2 chains)
            for tsb in range(4):          # 512-token superblocks (X^T panels)
                for tgl in range(2):  # groups of 2 t-tiles (256 tokens)
                    tg = tsb * 2 + tgl
                    ps_a = psa.tile([P, 2, 512], F32, tag="psa")
                    for ttl in range(2):
                        emit_chain(ps_a, ttl, x8_sb, xr_sb, tsb, tgl)
                    if len(pending) >= 2:
                        post_transpose(*pending.pop(0))
                    qkn = post_rope(tg, tgl, ps_a, ab_sb)
                    pending.append((tg, qkn))
            for item in pending:
                post_transpose(*item)

        # ---------------- Phase B: attention + o_proj -----------------------
        with ExitStack() as pb_ctx:
            expp = pb_ctx.enter_context(tc.tile_pool(name="expp", bufs=8))
            nrm = pb_ctx.enter_context(tc.tile_pool(name="nrm", bufs=2))
            esq = pb_ctx.enter_context(tc.tile_pool(name="esq", bufs=4))
            outp = pb_ctx.enter_context(tc.tile_pool(name="outp", bufs=4))
            pss = pb_ctx.enter_context(tc.tile_pool(name="pss", bufs=3, space="PSUM"))
            psat = pb_ctx.enter_context(tc.tile_pool(name="psat", bufs=2, space="PSUM"))
            psse = pb_ctx.enter_context(tc.tile_pool(name="psse", bufs=1, space="PSUM"))
            pso = pb_ctx.enter_context(tc.tile_pool(name="pso", bufs=2, space="PSUM"))

            out_tiles = {}
            flush_ctr = [0]

            def emit_oproj_unit(tt, hb, flush=False, eng="mix"):
                if hb == 0:
                    out_tiles[tt] = outp.tile(
                        [P, H], F16, tag="osb", name=f"osb_{tt}"
                    )
                o_sb = out_tiles[tt]
                if flush:
                    # during a flush the attention PSUM pools are idle:
                    # rotate across them so the PE never waits on a copy.
                    # mid-run flushes avoid psat (normalize still reads it).
                    pools = ((pso, "o"), (pss, "st"), (psat, "at"))[:flush]
                    pool, tg = pools[flush_ctr[0] % len(pools)]
                    flush_ctr[0] += 1
                    ps_o = pool.tile(
                        [P, 512], F32, tag=tg, name=f"of_{tt}_{hb}"
                    )
                else:
                    ps_o = pso.tile([P, 512], F32, tag="o")
                tcol = slice(tt * 128, (tt + 1) * 128)
                hcol = slice(hb * 512, (hb + 1) * 512)
                if tt >= 12:
                    # last t-block: plain fp16 o_proj (attn16 is ready right
                    # after the normalize mul -- no fp8 split on the tail)
                    lc = slice((tt - 12) * 128, (tt - 11) * 128)
                    for hh in range(2):
                        nc.tensor.matmul(
                            ps_o[:],
                            attn16[:, hh, lc],
                            wo16_sb[:, hh, hcol],
                            start=(hh == 0),
                            stop=(hh == 1),
                        )
                    oscale = 1.0 / 16.0  # attn carried 16x, wo16 exact
                else:
                    for i, (a, w) in enumerate(
                        ((attn8, wo8_sb), (attnr, wo8_sb), (attn8, wor_sb))
                    ):
                        nc.tensor.matmul(
                            ps_o[:],
                            a[:, :, tcol],
                            w[:, :, hcol],
                            start=(i == 0),
                            stop=(i == 2),
                            perf_mode=DR,
                        )
                    oscale = 1.0 / 256.0  # attn carried 16x, Wo carried 16x
                if (flush and hb % 2 == 0) or (not flush and eng == "mix" and hb % 2 == 0):
                    nc.scalar.activation(
                        o_sb[:, hcol], ps_o[:], AF.Copy, scale=oscale
                    )
                else:
                    nc.vector.tensor_scalar_mul(o_sb[:, hcol], ps_o[:], oscale)
                if hb == 3:
                    nc.sync.dma_start(
                        out[tt * 128:(tt + 1) * 128, :], o_sb[:]
                    )

            # o_proj units of t-block tbo, woven into the next attention
            # t-block's PE stream
            oproj_queue = []

            def queue_oproj(tbo):
                for ttl in range(4):
                    for hb in range(4):
                        oproj_queue.append((4 * tbo + ttl, hb))

            def normalize(qh, c0, width, ps_at, bc, fast=False):
                # attn_full = 16*attn (ones=1/16 in sum-exp); split into
                # e4m3 main + e5m2 residual for the fp8 o_proj
                s_ps = slice(c0 % 512, c0 % 512 + width)
                s_at = slice(c0, c0 + width)
                if fast:
                    # last t-block: straight to the fp16 o_proj operand
                    nc.vector.tensor_mul(
                        attn16[:, qh, s_ps], ps_at[:, s_ps], bc[:, s_ps]
                    )
                    return
                afull = nrm.tile([P, 512], F16, tag="afull")
                nc.vector.tensor_mul(
                    afull[:, 0:width], ps_at[:, s_ps], bc[:, s_ps]
                )
                nc.vector.tensor_copy(attn8[:, qh, s_at], afull[:, 0:width])
                nc.vector.tensor_sub(
                    attnr[:, qh, s_at], afull[:, 0:width], attn8[:, qh, s_at]
                )

            for tb in range(N_TB):
                qrhs0 = tb * 512
                nsb = 4 * (tb + 1)    # s-blocks of 128 tokens
                # diagonal (masked) s-blocks first (their longer
                # S^T -> mask -> exp chain pipelines over later blocks);
                # the two q heads interleave so av/se for head A fill the
                # PE while head B's exp chain cooks (and vice versa).
                diag = list(range(4 * tb, 4 * tb + 4))
                rest = list(range(4 * tb))
                sb_order = []
                for i in range(max(len(diag), len(rest))):
                    if i < len(diag):
                        sb_order.append(diag[i])
                    if i < len(rest):
                        sb_order.append(rest[i])

                ps_at = [psat.tile([P, 512], F32, tag="at", name=f"at{q}")
                         for q in range(2)]
                # one shared [P,512] bank: head 0 sums on partition 0,
                # head 1 on partition 32 (matmul outputs must start at a
                # multiple of 32)
                ps_se = psse.tile([P, 512], F32, tag="se")
                se_row = {0: 0, 1: 64}
                first_se = {0: True, 1: True}
                use_dr_se = False  # Pool quantize latency eats the PE win
                # countdown of SE matmuls: 4 diag per head + 1 joint DR (or
                # 2 fp16) per off-diag block
                se_left = [8 + (4 * tb if use_dr_se else 8 * tb)]
                es8_map = {}

                def emit_av(qh, sb, si, o):
                    es = es_tiles[(qh, sb)]
                    r = se_row[qh]
                    nc.tensor.matmul(
                        ps_at[qh][:, o:512],
                        v_tok[:, sb, :],
                        es[:, o:512],
                        start=(si == 0),
                        stop=(si == nsb - 1),
                        skip_group_check=True,
                    )
                    if sb >= 4 * tb or not use_dr_se:
                        # diagonal: exact fp16 sum-exp (keeps early rows'
                        # denominators quantization-free)
                        se_left[0] -= 1
                        nc.tensor.matmul(
                            ps_se[r:r + 1, o:512],
                            ones_sb[:],
                            es[:, o:512],
                            start=first_se[qh],
                            stop=(se_left[0] == 0),
                            skip_group_check=True,
                        )
                        first_se[qh] = False
                    elif qh == 1:
                        # off-diagonal: one joint DR matmul sums both heads'
                        # 128 s-rows (rows 1-31/65-95 accumulate garbage on
                        # never-started psum; only rows 0 and 64 are read)
                        pair = es8_map[sb]
                        se_left[0] -= 1
                        nc.tensor.matmul(
                            ps_se[:, :],
                            ones8_sb[:],
                            pair[:],
                            start=False,
                            stop=(se_left[0] == 0),
                            perf_mode=DR,
                            skip_group_check=True,
                        )

                es_tiles = {}
                pending = []
                for si, sb in enumerate(sb_order):
                    off = sb - 4 * tb
                    # diagonal blocks: columns left of the triangle are
                    # fully causally masked -- skip them outright
                    o = 128 * off if 0 <= off < 4 else 0
                    for qh in range(2):
                        ps_s = pss.tile([P, 512], F32, tag="st")
                        nc.tensor.matmul(
                            ps_s[:, o:512],
                            qT[:, 2, sb * 128:(sb + 1) * 128],
                            qT[:, qh, qrhs0 + o:qrhs0 + 512],
                            start=True,
                            stop=True,
                        )
                        if 0 <= off < 4:
                            nc.vector.tensor_add(
                                ps_s[:, o:o + 128], ps_s[:, o:o + 128],
                                mask_sb[:, off, o:o + 128],
                            )
                        es = expp.tile([P, 512], F16, tag="es")
                        es_tiles[(qh, sb)] = es
                        nc.scalar.activation(
                            es[:, o:512], ps_s[:, o:512], AF.Exp, scale=SCALE
                        )
                        if sb < 4 * tb and use_dr_se:
                            if qh == 0:
                                es8_map[sb] = esq.tile(
                                    [P, 2, 512], F8E4, tag="es8",
                                    name=f"es8_{tb}_{si}",
                                )
                            nc.gpsimd.tensor_copy(
                                es8_map[sb][:, qh, :], es[:]
                            )
                        pending.append((qh, sb, si, o))
                        if len(pending) > 4:
                            emit_av(*pending.pop(0))
                    if oproj_queue:
                        weng = "mix" if tb == 1 else "dve"
                        emit_oproj_unit(*oproj_queue.pop(0), eng=weng)
                        if len(oproj_queue) > 10:
                            emit_oproj_unit(*oproj_queue.pop(0), eng=weng)
                def finish_head(qh):
                    r = se_row[qh]
                    inv_sb = nrm.tile([1, 512], F32, tag="inv")
                    nc.vector.reciprocal(inv_sb[:], ps_se[r:r + 1, :])
                    bc = nrm.tile([P, 512], F32, tag="bc")
                    nc.gpsimd.partition_broadcast(bc[:], inv_sb[0:1, :])
                    if tb == N_TB - 1:
                        for ttn in range(4):
                            normalize(qh, qrhs0 + ttn * 128, 128, ps_at[qh],
                                      bc, fast=True)
                    else:
                        normalize(qh, qrhs0, 512, ps_at[qh], bc)

                done_heads = set()
                for item in pending:
                    emit_av(*item)
                    qh_i = item[0]
                    if all(p[0] != qh_i for p in pending[pending.index(item) + 1:]):
                        if qh_i not in done_heads:
                            done_heads.add(qh_i)
                            finish_head(qh_i)
                for qh_i in range(2):
                    if qh_i not in done_heads:
                        finish_head(qh_i)
                while oproj_queue:
                    emit_oproj_unit(*oproj_queue.pop(0), flush=1)
                queue_oproj(tb)
            while oproj_queue:
                emit_oproj_unit(*oproj_queue.pop(0), flush=3)

    nc.compile()
    return nc


_NC_CACHE = {}


def _get_nc():
    if "nc" not in _NC_CACHE:
        _NC_CACHE["nc"] = _build_nc()
    return _NC_CACHE["nc"]


def _fp8_split(x, scale):
    """Split scale*x into an e4m3 main part (subnormals flushed so host and
    PE agree) and an e5m2 residual."""
    import ml_dtypes

    xs = np.asarray(x, dtype=np.float32) * scale
    hi = xs.astype(ml_dtypes.float8_e4m3).astype(np.float32)
    hi[np.abs(hi) < 2.0 ** -6] = 0.0
    lo = (xs - hi).astype(ml_dtypes.float8_e5m2)
    return hi.astype(ml_dtypes.float8_e4m3), lo


def kernel(positions, hidden_states, Wqkv, Wo, q_norm_w, k_norm_w):
    positions = np.asarray(positions)
    out_dtype = np.asarray(hidden_states).dtype
    hs = np.asarray(hidden_states, dtype=np.float32)
    Wqkv = np.asarray(Wqkv, dtype=np.float32)
    Wo = np.asarray(Wo, dtype=np.float32)
    qw = np.asarray(q_norm_w, dtype=np.float32)
    kw = np.asarray(k_norm_w, dtype=np.float32)

    # ----- host-side input prep -----
    xt = np.ascontiguousarray(hs.T)
    x8_h, xr_h = _fp8_split(xt, 4.0)

    inv_freq = (1.0 / (THETA ** (np.arange(0, HD, 2, dtype=np.float32) / HD))).astype(
        np.float32
    )
    freqs = positions.astype(np.float32)[:, None] * inv_freq[None, :]  # [T, 64]
    cos = np.cos(freqs).astype(np.float32)
    sin = np.sin(freqs).astype(np.float32)

    def ab_tables(wvec):
        a1 = cos * wvec[None, :64]
        b1 = sin * wvec[None, 64:]
        a2 = cos * wvec[None, 64:]
        b2 = sin * wvec[None, :64]
        return np.stack([a1, b1, a2, b2], axis=1)  # [T, 4, 64]

    abq = ab_tables(qw)
    abk = ab_tables(kw)
    # combined per-head tables, head order (q0, q1, k)
    ab = np.ascontiguousarray(
        np.stack([abq, abq, abk], axis=2), dtype=np.float16
    )  # [T, 4, 3, 64]

    # causal masks for the four diagonal 128-row groups of each 512-col t-block
    t_in = np.arange(512)
    s_in = np.arange(128)
    maskc = np.empty((P, 4, 512), dtype=np.float16)
    for j in range(4):
        maskc[:, j, :] = np.where(
            (j * 128 + s_in)[:, None] <= t_in[None, :], 0.0, NEG
        )
    ident = np.eye(P, dtype=np.float16)

    q_size = N_HEADS * HD
    kv_size = N_KV * HD
    in_maps = []
    for j in range(N_CORES):
        qs = slice(2 * j * HD, (2 * j + 2) * HD)
        ks = slice(q_size + j * HD, q_size + (j + 1) * HD)
        vs = slice(q_size + kv_size + j * HD, q_size + kv_size + (j + 1) * HD)
        wj = np.ascontiguousarray(
            np.concatenate([Wqkv[:, qs], Wqkv[:, ks], Wqkv[:, vs]], axis=1)
        )
        w8_h, wr_h = _fp8_split(wj, 32.0)
        wo8_h, wor_h = _fp8_split(Wo[qs, :], 16.0)
        in_maps.append(
            {
                "x8": x8_h,
                "xr": xr_h,
                "w8": w8_h,
                "wr": wr_h,
                "wo8": wo8_h,
                "wor": wor_h,
                "wo16": np.ascontiguousarray(Wo[qs, :]).astype(np.float16),
                "ab": ab,
                "maskc": maskc,
                "ident": ident,
            }
        )

    nc = _get_nc()
    res = run_bass_kernel_spmd(nc, in_maps, core_ids=list(range(N_CORES)))

    acc = res.results[0]["out"].astype(np.float32)
    for j in range(1, N_CORES):
        acc += res.results[j]["out"].astype(np.float32)
    return acc.astype(out_dtype, copy=False)


# revision 46
# speedup vs baseline: 1.0075x; 1.0005x over previous
"""Trainium2 Bass kernel for ConvGPTAttention (dense transformer attention block).

Sharding: tensor-parallel by head groups across 8 NeuronCores.
Core j owns q heads {2j, 2j+1} and kv head j (GQA maps q head h -> kv head h//2,
so each core's attention is fully local). Wqkv is column-sharded, Wo is
row-sharded; the 8 partial o_proj outputs are summed on the host (the
"all-reduce" of RowParallelLinear, done at unshard time).

Per-core pipeline:
  Phase A: qkv = X @ Wqkv_shard via fp8 DoubleRow matmuls (2 cols/PE-cycle),
           3-term error compensation: X4=4X, W32=32W split into e4m3 main +
           e5m2 residual; qkv = X8@W8 + Xr@W8 + X8@Wr, PSUM carries 128x the
           true value and the post-copies fold in 1/128. Per-head RMSNorm
           stats via tensor_tensor_reduce (DVE), neox RoPE on fp16 operands
           (2x DVE modes), PE transposes of q/k in fp16, all matmul operands
           in fp16 thereafter.
  Phase B: causal attention per (q head, 512-col t-block) in S^T layout at
           s-block (128-token) granularity; exp on ACT (softmax scale folded),
           av/sum-exp matmuls lag two s-blocks behind S^T; sum-exp ones
           vector holds 1/16 so attn is normalized to 16*attn (keeps the
           on-device e4m3 split of attn out of the subnormal range).
           o_proj in fp8 DoubleRow with 3-term compensation (attn8/attnr
           split on DVE, Wo8/Wor split on host, output copies scale 1/256);
           o_proj partial outputs are batched per 128-token row block into
           single DMAs.
"""

import numpy as np
from contextlib import ExitStack

import concourse.bacc as bacc
import concourse.mybir as mybir
import concourse.tile as tile
from concourse.bass_utils import run_bass_kernel_spmd

P = 128
T = 2048
H = 2048
N_HEADS = 16
N_KV = 8
HD = 128
EPS = 1e-6
THETA = 10000.0
SCALE = HD ** -0.5
NEG = -60000.0  # additive mask (fp16-safe); SCALE*NEG = -5303 -> exp == 0.0

F32 = mybir.dt.float32
F16 = mybir.dt.float16
F8E4 = mybir.dt.float8e4
F8E5 = mybir.dt.float8e5
AF = mybir.ActivationFunctionType
ALU = mybir.AluOpType
DR = mybir.MatmulPerfMode.DoubleRow

N_CORES = 8
N_TT = 16        # t-tiles of 128 tokens
N_TB = 4         # t-blocks of 512 tokens (attention rhs width)


def _build_nc():
    nc = bacc.Bacc("TRN2", target_bir_lowering=False, debug=False)

    x8 = nc.dram_tensor("x8", [H, T], F8E4, kind="ExternalInput")
    xr = nc.dram_tensor("xr", [H, T], F8E5, kind="ExternalInput")
    w8 = nc.dram_tensor("w8", [H, 512], F8E4, kind="ExternalInput")
    wr = nc.dram_tensor("wr", [H, 512], F8E5, kind="ExternalInput")
    wo8 = nc.dram_tensor("wo8", [256, H], F8E4, kind="ExternalInput")
    wor = nc.dram_tensor("wor", [256, H], F8E5, kind="ExternalInput")
    ab = nc.dram_tensor("ab", [T, 4, 3, 64], F16, kind="ExternalInput")
    maskc = nc.dram_tensor("maskc", [P, 4, 512], F16, kind="ExternalInput")
    ident = nc.dram_tensor("ident", [P, P], F16, kind="ExternalInput")
    wo16 = nc.dram_tensor("wo16", [256, H], F16, kind="ExternalInput")
    out = nc.dram_tensor("out", [T, H], F16, kind="ExternalOutput")

    with ExitStack() as top:
        tc = top.enter_context(tile.TileContext(nc))
        pers = top.enter_context(tc.tile_pool(name="pers", bufs=1))

        mask_sb = pers.tile([P, 4, 512], F16, tag="maskc")
        ident_sb = pers.tile([P, P], F16, tag="ident")
        ones_sb = pers.tile([P, 1], F16, tag="ones")
        nc.vector.memset(ones_sb[:], 1.0 / 16.0)  # folds 1/16 into sum-exp
        # DoubleRow matmuls must target PSUM partition 0, so one joint DR
        # sum-exp serves both heads: slot 0 = head0 es, slot 1 = head1 es,
        # and a structured ones matrix routes slot 0 sums to rows [0,32)
        # and slot 1 sums to rows [64,96). Only rows 0 and 64 are read.
        ones8_sb = pers.tile([P, 2, P], F8E4, tag="ones8")
        nc.vector.memset(ones8_sb[:], 0.0)
        nc.vector.memset(ones8_sb[:, 0, 0:32], 1.0 / 16.0)  # 2^-4: exact
        nc.vector.memset(ones8_sb[:, 1, 64:96], 1.0 / 16.0)
        eps_sb = pers.tile([P, 1], F32, tag="eps")
        nc.vector.memset(eps_sb[:], EPS)

        # persistent activations (all fp16 matmul operands)
        qT = pers.tile([P, 3, T], F16, tag="qT")        # [d, (q0,q1,k), t]
        v_tok = pers.tile([P, N_TT, P], F16, tag="v")   # [t_in, tt, d]
        attn8 = pers.tile([P, 2, T], F8E4, tag="attn8")  # 16*attn, e4m3 part
        attnr = pers.tile([P, 2, T], F8E5, tag="attnr")  # residual
        w8_sb = pers.tile([P, 16, 512], F8E4, tag="w8")
        wr_sb = pers.tile([P, 16, 512], F8E5, tag="wr")
        wo8_sb = pers.tile([P, 2, H], F8E4, tag="wo8")
        wor_sb = pers.tile([P, 2, H], F8E5, tag="wor")
        wo16_sb = pers.tile([P, 2, H], F16, tag="wo16")
        attn16 = pers.tile([P, 2, 512], F16, tag="attn16")  # tb=3: 16*attn fp16

        # ---------------- Phase A: QKV + norm + rope + transpose ------------
        with ExitStack() as pa_ctx:
            xtp = pa_ctx.enter_context(tc.tile_pool(name="xtp", bufs=1))
            pa = pa_ctx.enter_context(tc.tile_pool(name="pa", bufs=3))
            psa = pa_ctx.enter_context(tc.tile_pool(name="psa", bufs=3, space="PSUM"))
            pst = pa_ctx.enter_context(tc.tile_pool(name="pst", bufs=2, space="PSUM"))

            scr = xtp.tile([P, 2, 512], F8E4, tag="scr")
            nc.vector.memset(scr[:], 0.0)
            ps_w = psa.tile([P, 2, 512], F32, tag="psa", name="ps_warm")
            for i in range(12):
                nc.tensor.matmul(
                    ps_w[:, i % 2, :],
                    scr[:, :, 0:128],
                    scr[:],
                    start=True,
                    stop=True,
                    perf_mode=DR,
                    skip_group_check=True,
                )

            # X panels live in SBUF for the whole phase (64 KiB/partition):
            # every DMA is issued once, up front, across both HWDGE queues
            x8_sb = xtp.tile([P, 16, T], F8E4, tag="x8")
            xr_sb = xtp.tile([P, 16, T], F8E5, tag="xr")
            ab_sb = xtp.tile([P, 16, 4, 3, 64], F16, tag="ab")

            def dma_w(dst, src, chunks):
                step = 16 // chunks
                for c in range(chunks):
                    nc.sync.dma_start(
                        dst[:, c * step:(c + 1) * step, :],
                        src.rearrange("(hi p) n -> p hi n", p=P)[
                            :, c * step:(c + 1) * step, :
                        ],
                    )

            def dma_x(dst, src, tsb, chunks):
                step = 16 // chunks
                srcv = src.rearrange("(hi p) n -> p hi n", p=P)
                cols = slice(tsb * 512, (tsb + 1) * 512)
                for c in range(chunks):
                    nc.sync.dma_start(
                        dst[:, c * step:(c + 1) * step, cols],
                        srcv[:, c * step:(c + 1) * step, cols],
                    )

            def post_rope(tg, tgl, ps_a, ab_sb):
                # views on the psum group: [P, 2(ttl), 4(head), 128(d)]
                # psum holds 128*qkv (X scaled 4x, W scaled 32x)
                ps_r = ps_a.rearrange("p g (h d) -> p g h d", h=4)

                # raw q/k in fp16 (scale back), v straight to persistent
                qkn_raw = pa.tile([P, 2, 3, 128], F16, tag="qkraw")
                nc.scalar.activation(
                    qkn_raw[:], ps_r[:, :, 0:3, :], AF.Copy, scale=1.0 / 128.0
                )
                nc.scalar.activation(
                    v_tok[:, 2 * tg:2 * tg + 2, :], ps_r[:, :, 3, :], AF.Copy,
                    scale=1.0 / 128.0,
                )

                # RMS stats: square (DVE, 2x fp16) -> free-axis reduce
                sq = pa.tile([P, 2, 3, 128], F16, tag="sq")
                nc.vector.tensor_mul(sq[:], qkn_raw[:], qkn_raw[:])
                ss = pa.tile([P, 2, 3], F32, tag="ss")
                nc.vector.tensor_reduce(
                    ss[:], sq[:], axis=mybir.AxisListType.X, op=ALU.add
                )
                sr = pa.tile([P, 2, 3], F32, tag="sr")
                nc.scalar.activation(
                    sr[:], ss[:], AF.Sqrt, scale=1.0 / HD, bias=eps_sb[:]
                )
                s_inv = pa.tile([P, 2, 3], F32, tag="si")
                nc.vector.reciprocal(s_inv[:], sr[:])

                # rope (tables have norm weight folded in, heads packed
                # (q0, q1, k) along the table's head dim):
                # out1 = x1*a1 - x2*b1 ; out2 = x2*a2 + x1*b2
                qkn = pa.tile([P, 2, 3, 128], F16, tag="qkn")
                x1 = qkn_raw[:, :, :, 0:64]
                x2 = qkn_raw[:, :, :, 64:128]
                abg = ab_sb[:, 2 * tg:2 * tg + 2]    # [P, 2, 4, 3, 64]
                m1 = pa.tile([P, 2, 3, 64], F16, tag="m1")
                m2 = pa.tile([P, 2, 3, 64], F16, tag="m2")
                nc.vector.tensor_mul(m1[:], x1, abg[:, :, 0])
                nc.vector.tensor_mul(m2[:], x2, abg[:, :, 1])
                nc.vector.tensor_sub(qkn[:, :, :, 0:64], m1[:], m2[:])
                nc.vector.tensor_mul(m1[:], x2, abg[:, :, 2])
                nc.vector.tensor_mul(m2[:], x1, abg[:, :, 3])
                nc.vector.tensor_add(qkn[:, :, :, 64:128], m1[:], m2[:])

                # apply 1/rms (per token+head scalar, 2x DVE mode kept)
                for k in range(6):
                    ttl, h = divmod(k, 3)
                    nc.vector.tensor_scalar_mul(
                        qkn[:, ttl, h, :], qkn[:, ttl, h, :],
                        s_inv[:, ttl, h:h + 1],
                    )
                return qkn

            def post_transpose(tg, qkn):
                # transpose q0/q1/k to [d, t] (fp16 transposes, 1 cyc/row)
                for ttl in range(2):
                    tt = 2 * tg + ttl
                    ps_t = pst.tile([P, 3, P], F16, tag="pst")
                    for h in range(3):
                        nc.tensor.transpose(
                            ps_t[:, h, :], qkn[:, ttl, h, :], ident_sb[:]
                        )
                    nc.vector.tensor_copy(
                        qT[:, :, tt * 128:(tt + 1) * 128], ps_t[:]
                    )

            def emit_chain(ps_a, ttl, x8_sb, xr_sb, tsb, tgl):
                c0 = tsb * 512 + tgl * 256 + ttl * 128
                col = slice(c0, c0 + 128)
                idx = 0
                for xs, ws in ((x8_sb, w8_sb), (x8_sb, wr_sb), (xr_sb, w8_sb)):
                    for j in range(8):
                        nc.tensor.matmul(
                            ps_a[:, ttl, :],
                            xs[:, 2 * j:2 * j + 2, col],
                            ws[:, 2 * j:2 * j + 2, :],
                            start=(idx == 0),
                            stop=(idx == 23),
                            perf_mode=DR,
                        )
                        idx += 1

            # --- all input DMAs up front, on the SP queue, ordered by
            # first-consumption time (issue cadence is ~0.9us/DMA, so the
            # order IS the arrival schedule)
            w8v = w8.rearrange("(hi p) n -> p hi n", p=P)
            x8v = x8.rearrange("(hi p) n -> p hi n", p=P)
            abv = ab.rearrange("(tt p) f h d -> p tt f h d", p=P)
            for c in range(4):
                hs = slice(c * 4, (c + 1) * 4)
                nc.sync.dma_start(w8_sb[:, hs, :], w8v[:, hs, :])
                nc.sync.dma_start(x8_sb[:, hs, 0:512], x8v[:, hs, 0:512])
            dma_w(wr_sb, wr, 2)
            dma_x(xr_sb, xr, 0, 2)
            nc.sync.dma_start(ab_sb[:, 0:8], abv[:, 0:8])
            dma_x(x8_sb, x8, 1, 1)
            dma_x(xr_sb, xr, 1, 1)
            nc.sync.dma_start(ab_sb[:, 8:16], abv[:, 8:16])
            dma_x(x8_sb, x8, 2, 1)
            dma_x(xr_sb, xr, 2, 1)
            nc.sync.dma_start(ident_sb[:], ident[:])
            dma_x(x8_sb, x8, 3, 1)
            dma_x(xr_sb, xr, 3, 1)
            nc.sync.dma_start(
                wo8_sb[:], wo8.rearrange("(do p) h -> p do h", p=P)
            )
            nc.sync.dma_start(
                wor_sb[:], wor.rearrange("(do p) h -> p do h", p=P)
            )
            nc.sync.dma_start(
                wo16_sb[:], wo16.rearrange("(do p) h -> p do h", p=P)
            )
            nc.sync.dma_start(mask_sb[:], maskc[:])

            pending = []  # (tg, qkn) awaiting transposes (lag 2 chains)
            for tsb in range(4):          # 512-token superblocks (X^T panels)
                for tgl in range(2):  # groups of 2 t-tiles (256 tokens)
                    tg = tsb * 2 + tgl
                    ps_a = psa.tile([P, 2, 512], F32, tag="psa")
                    for ttl in range(2):
                        emit_chain(ps_a, ttl, x8_sb, xr_sb, tsb, tgl)
                    if len(pending) >= 2:
                        post_transpose(*pending.pop(0))
                    qkn = post_rope(tg, tgl, ps_a, ab_sb)
                    pending.append((tg, qkn))
            for item in pending:
                post_transpose(*item)

        # ---------------- Phase B: attention + o_proj -----------------------
        with ExitStack() as pb_ctx:
            expp = pb_ctx.enter_context(tc.tile_pool(name="expp", bufs=8))
            nrm = pb_ctx.enter_context(tc.tile_pool(name="nrm", bufs=2))
            esq = pb_ctx.enter_context(tc.tile_pool(name="esq", bufs=4))
            outp = pb_ctx.enter_context(tc.tile_pool(name="outp", bufs=4))
            pss = pb_ctx.enter_context(tc.tile_pool(name="pss", bufs=3, space="PSUM"))
            psat = pb_ctx.enter_context(tc.tile_pool(name="psat", bufs=2, space="PSUM"))
            psse = pb_ctx.enter_context(tc.tile_pool(name="psse", bufs=1, space="PSUM"))
            pso = pb_ctx.enter_context(tc.tile_pool(name="pso", bufs=2, space="PSUM"))

            out_tiles = {}
            flush_ctr = [0]

            def emit_oproj_unit(tt, hb, flush=False, eng="mix"):
                if hb == 0:
                    out_tiles[tt] = outp.tile(
                        [P, H], F16, tag="osb", name=f"osb_{tt}"
                    )
                o_sb = out_tiles[tt]
                if flush:
                    # during a flush the attention PSUM pools are idle:
                    # rotate across them so the PE never waits on a copy.
                    # mid-run flushes avoid psat (normalize still reads it).
                    pools = ((pso, "o"), (pss, "st"), (psat, "at"))[:flush]
                    pool, tg = pools[flush_ctr[0] % len(pools)]
                    flush_ctr[0] += 1
                    ps_o = pool.tile(
                        [P, 512], F32, tag=tg, name=f"of_{tt}_{hb}"
                    )
                else:
                    ps_o = pso.tile([P, 512], F32, tag="o")
                tcol = slice(tt * 128, (tt + 1) * 128)
                hcol = slice(hb * 512, (hb + 1) * 512)
                if tt >= 12:
                    # last t-block: plain fp16 o_proj (attn16 is ready right
                    # after the normalize mul -- no fp8 split on the tail)
                    lc = slice((tt - 12) * 128, (tt - 11) * 128)
                    for hh in range(2):
                        nc.tensor.matmul(
                            ps_o[:],
                            attn16[:, hh, lc],
                            wo16_sb[:, hh, hcol],
                            start=(hh == 0),
                            stop=(hh == 1),
                        )
                    oscale = 1.0 / 16.0  # attn carried 16x, wo16 exact
                else:
                    for i, (a, w) in enumerate(
                        ((attn8, wo8_sb), (attnr, wo8_sb), (attn8, wor_sb))
                    ):
                        nc.tensor.matmul(
                            ps_o[:],
                            a[:, :, tcol],
                            w[:, :, hcol],
                            start=(i == 0),
                            stop=(i == 2),
                            perf_mode=DR,
                        )
                    oscale = 1.0 / 256.0  # attn carried 16x, Wo carried 16x
                if (flush and hb % 2 == 0) or (not flush and eng == "mix" and hb % 2 == 0):
                    nc.scalar.activation(
                        o_sb[:, hcol], ps_o[:], AF.Copy, scale=oscale
                    )
                else:
                    nc.vector.tensor_scalar_mul(o_sb[:, hcol], ps_o[:], oscale)
                if hb == 3:
                    nc.sync.dma_start(
                        out[tt * 128:(tt + 1) * 128, :], o_sb[:]
                    )

            # o_proj units of t-block tbo, woven into the next attention
            # t-block's PE stream
            oproj_queue = []

            def queue_oproj(tbo):
                for ttl in range(4):
                    for hb in range(4):
                        oproj_queue.append((4 * tbo + ttl, hb))

            def normalize(qh, c0, width, ps_at, bc, fast=False):
                # attn_full = 16*attn (ones=1/16 in sum-exp); split into
                # e4m3 main + e5m2 residual for the fp8 o_proj
                s_ps = slice(c0 % 512, c0 % 512 + width)
                s_at = slice(c0, c0 + width)
                if fast:
                    # last t-block: straight to the fp16 o_proj operand
                    nc.vector.tensor_mul(
                        attn16[:, qh, s_ps], ps_at[:, s_ps], bc[:, s_ps]
                    )
                    return
                afull = nrm.tile([P, 512], F16, tag="afull")
                nc.vector.tensor_mul(
                    afull[:, 0:width], ps_at[:, s_ps], bc[:, s_ps]
                )
                nc.vector.tensor_copy(attn8[:, qh, s_at], afull[:, 0:width])
                nc.vector.tensor_sub(
                    attnr[:, qh, s_at], afull[:, 0:width], attn8[:, qh, s_at]
                )

            for tb in range(N_TB):
                qrhs0 = tb * 512
                nsb = 4 * (tb + 1)    # s-blocks of 128 tokens
                # diagonal (masked) s-blocks first (their longer
                # S^T -> mask -> exp chain pipelines over later blocks);
                # the two q heads interleave so av/se for head A fill the
                # PE while head B's exp chain cooks (and vice versa).
                diag = list(range(4 * tb, 4 * tb + 4))
                rest = list(range(4 * tb))
                sb_order = []
                for i in range(max(len(diag), len(rest))):
                    if i < len(diag):
                        sb_order.append(diag[i])
                    if i < len(rest):
                        sb_order.append(rest[i])

                ps_at = [psat.tile([P, 512], F32, tag="at", name=f"at{q}")
                         for q in range(2)]
                # one shared [P,512] bank: head 0 sums on partition 0,
                # head 1 on partition 32 (matmul outputs must start at a
                # multiple of 32)
                ps_se = psse.tile([P, 512], F32, tag="se")
                se_row = {0: 0, 1: 64}
                first_se = {0: True, 1: True}
                use_dr_se = False  # Pool quantize latency eats the PE win
                # countdown of SE matmuls: 4 diag per head + 1 joint DR (or
                # 2 fp16) per off-diag block
                se_left = [8 + (4 * tb if use_dr_se else 8 * tb)]
                es8_map = {}

                def emit_av(qh, sb, si, o):
                    es = es_tiles[(qh, sb)]
                    r = se_row[qh]
                    nc.tensor.matmul(
                        ps_at[qh][:, o:512],
                        v_tok[:, sb, :],
                        es[:, o:512],
                        start=(si == 0),
                        stop=(si == nsb - 1),
                        skip_group_check=True,
                    )
                    if sb >= 4 * tb or not use_dr_se:
                        # diagonal: exact fp16 sum-exp (keeps early rows'
                        # denominators quantization-free)
                        se_left[0] -= 1
                        nc.tensor.matmul(
                            ps_se[r:r + 1, o:512],
                            ones_sb[:],
                            es[:, o:512],
                            start=first_se[qh],
                            stop=(se_left[0] == 0),
                            skip_group_check=True,
                        )
                        first_se[qh] = False
                    elif qh == 1:
                        # off-diagonal: one joint DR matmul sums both heads'
                        # 128 s-rows (rows 1-31/65-95 accumulate garbage on
                        # never-started psum; only rows 0 and 64 are read)
                        pair = es8_map[sb]
                        se_left[0] -= 1
                        nc.tensor.matmul(
                            ps_se[:, :],
                            ones8_sb[:],
                            pair[:],
                            start=False,
                            stop=(se_left[0] == 0),
                            perf_mode=DR,
                            skip_group_check=True,
                        )

                es_tiles = {}
                pending = []
                for si, sb in enumerate(sb_order):
                    off = sb - 4 * tb
                    # diagonal blocks: columns left of the triangle are
                    # fully causally masked -- skip them outright
                    o = 128 * off if 0 <= off < 4 else 0
                    for qh in range(2):
                        ps_s = pss.tile([P, 512], F32, tag="st")
                        nc.tensor.matmul(
                            ps_s[:, o:512],
                            qT[:, 2, sb * 128:(sb + 1) * 128],
                            qT[:, qh, qrhs0 + o:qrhs0 + 512],
                            start=True,
                            stop=True,
                        )
                        if 0 <= off < 4:
                            nc.vector.tensor_add(
                                ps_s[:, o:o + 128], ps_s[:, o:o + 128],
                                mask_sb[:, off, o:o + 128],
                            )
                        es = expp.tile([P, 512], F16, tag="es")
                        es_tiles[(qh, sb)] = es
                        nc.scalar.activation(
                            es[:, o:512], ps_s[:, o:512], AF.Exp, scale=SCALE
                        )
                        if sb < 4 * tb and use_dr_se:
                            if qh == 0:
                                es8_map[sb] = esq.tile(
                                    [P, 2, 512], F8E4, tag="es8",
                                    name=f"es8_{tb}_{si}",
                                )
                            nc.gpsimd.tensor_copy(
                                es8_map[sb][:, qh, :], es[:]
                            )
                        pending.append((qh, sb, si, o))
                        if len(pending) > 4:
                            emit_av(*pending.pop(0))
                    if oproj_queue:
                        weng = "mix" if tb == 1 else "dve"
                        emit_oproj_unit(*oproj_queue.pop(0), eng=weng)
                        if len(oproj_queue) > 10:
                            emit_oproj_unit(*oproj_queue.pop(0), eng=weng)
                def finish_head(qh):
                    r = se_row[qh]
                    inv_sb = nrm.tile([1, 512], F32, tag="inv")
                    nc.vector.reciprocal(inv_sb[:], ps_se[r:r + 1, :])
                    bc = nrm.tile([P, 512], F32, tag="bc")
                    nc.gpsimd.partition_broadcast(bc[:], inv_sb[0:1, :])
                    if tb == N_TB - 1:
                        for ttn in range(4):
                            normalize(qh, qrhs0 + ttn * 128, 128, ps_at[qh],
                                      bc, fast=True)
                    else:
                        normalize(qh, qrhs0, 512, ps_at[qh], bc)

                done_heads = set()
                for item in pending:
                    emit_av(*item)
                    qh_i = item[0]
                    if all(p[0] != qh_i for p in pending[pending.index(item) + 1:]):
                        if qh_i not in done_heads:
                            done_heads.add(qh_i)
                            finish_head(qh_i)
                for qh_i in range(2):
                    if qh_i not in done_heads:
                        finish_head(qh_i)
                while oproj_queue:
                    emit_oproj_unit(*oproj_queue.pop(0), flush=1)
                queue_oproj(tb)
            while oproj_queue:
                emit_oproj_unit(*oproj_queue.pop(0), flush=3)

    nc.compile()
    return nc


_NC_CACHE = {}


def _get_nc():
    if "nc" not in _NC_CACHE:
        _NC_CACHE["nc"] = _build_nc()
    return _NC_CACHE["nc"]


def _fp8_split(x, scale):
    """Split scale*x into an e4m3 main part (subnormals flushed so host and
    PE agree) and an e5m2 residual."""
    import ml_dtypes

    xs = np.asarray(x, dtype=np.float32) * scale
    hi = xs.astype(ml_dtypes.float8_e4m3).astype(np.float32)
    hi[np.abs(hi) < 2.0 ** -6] = 0.0
    lo = (xs - hi).astype(ml_dtypes.float8_e5m2)
    return hi.astype(ml_dtypes.float8_e4m3), lo


def kernel(positions, hidden_states, Wqkv, Wo, q_norm_w, k_norm_w):
    positions = np.asarray(positions)
    out_dtype = np.asarray(hidden_states).dtype
    hs = np.asarray(hidden_states, dtype=np.float32)
    Wqkv = np.asarray(Wqkv, dtype=np.float32)
    Wo = np.asarray(Wo, dtype=np.float32)
    qw = np.asarray(q_norm_w, dtype=np.float32)
    kw = np.asarray(k_norm_w, dtype=np.float32)

    # ----- host-side input prep -----
    xt = np.ascontiguousarray(hs.T)
    x8_h, xr_h = _fp8_split(xt, 4.0)

    inv_freq = (1.0 / (THETA ** (np.arange(0, HD, 2, dtype=np.float32) / HD))).astype(
        np.float32
    )
    freqs = positions.astype(np.float32)[:, None] * inv_freq[None, :]  # [T, 64]
    cos = np.cos(freqs).astype(np.float32)
    sin = np.sin(freqs).astype(np.float32)

    def ab_tables(wvec):
        a1 = cos * wvec[None, :64]
        b1 = sin * wvec[None, 64:]
        a2 = cos * wvec[None, 64:]
        b2 = sin * wvec[None, :64]
        return np.stack([a1, b1, a2, b2], axis=1)  # [T, 4, 64]

    abq = ab_tables(qw)
    abk = ab_tables(kw)
    # combined per-head tables, head order (q0, q1, k)
    ab = np.ascontiguousarray(
        np.stack([abq, abq, abk], axis=2), dtype=np.float16
    )  # [T, 4, 3, 64]

    # causal masks for the four diagonal 128-row groups of each 512-col t-block
    t_in = np.arange(512)
    s_in = np.arange(128)
    maskc = np.empty((P, 4, 512), dtype=np.float16)
    for j in range(4):
        maskc[:, j, :] = np.where(
            (j * 128 + s_in)[:, None] <= t_in[None, :], 0.0, NEG
        )
    ident = np.eye(P, dtype=np.float16)

    q_size = N_HEADS * HD
    kv_size = N_KV * HD
    in_maps = []
    for j in range(N_CORES):
        qs = slice(2 * j * HD, (2 * j + 2) * HD)
        ks = slice(q_size + j * HD, q_size + (j + 1) * HD)
        vs = slice(q_size + kv_size + j * HD, q_size + kv_size + (j + 1) * HD)
        wj = np.ascontiguousarray(
            np.concatenate([Wqkv[:, qs], Wqkv[:, ks], Wqkv[:, vs]], axis=1)
        )
        w8_h, wr_h = _fp8_split(wj, 32.0)
        wo8_h, wor_h = _fp8_split(Wo[qs, :], 16.0)
        in_maps.append(
            {
                "x8": x8_h,
                "xr": xr_h,
                "w8": w8_h,
                "wr": wr_h,
                "wo8": wo8_h,
                "wor": wor_h,
                "wo16": np.ascontiguousarray(Wo[qs, :]).astype(np.float16),
                "ab": ab,
                "maskc": maskc,
                "ident": ident,
            }
        )

    nc = _get_nc()
    res = run_bass_kernel_spmd(nc, in_maps, core_ids=list(range(N_CORES)))

    acc = res.results[0]["out"].astype(np.float32)
    for j in range(1, N_CORES):
        acc += res.results[j]["out"].astype(np.float32)
    return acc.astype(out_dtype, copy=False)
